# revision 74
# baseline (speedup 1.0000x reference)
"""Multi-head attention (16 heads, d_model=2048, seq=2048, causal) on 8 trn2 cores.

Sharding: tensor-parallel over heads (2 heads/core) for QKV projection and
attention; two per-head AllToAlls redistribute the (normalized) per-head
attention outputs so each core holds all heads for a 256-row query slice;
each core then runs the full output projection for its slice and the host
concatenates the 8 slices.

Structure (v3):
 - Phase 1 and attention are interleaved per head: pass 0 computes Q/K for
   head 0 plus V for both heads, then runs head-0 attention and triggers its
   AllToAll at ~45% of the kernel; pass 1 (Q/K for head 1) and head-1
   attention execute under that collective, hiding its rendezvous skew and
   transfer time entirely.
 - Attention is software-pipelined: score matmuls run 2 chunk-pairs ahead of
   the AV/denominator matmuls; exp is done on [128,2,512] pairs to amortize
   the scalar engine's fixed per-instruction overhead (scalar stays below PE).
 - Softmax denominator via a replicated-ones [128x128] stationary matmul into
   a [128,512] PSUM tile; normalization is a full-lane reciprocal_approx_fast
   + tensor_mul on the vector engine.
 - Causal mask applied structurally: strictly-upper 128x512 blocks skipped;
   the 4 distinct diagonal-block patterns are precomputed bf16 0/1 masks
   applied with vector tensor_mul (gpsimd stays free so AllToAlls trigger
   immediately).
 - All matmul operands bf16 (fp32 PSUM accumulation).
"""
import sys

sys.path.insert(0, "/opt/trn_rl_repo")

import numpy as np
import ml_dtypes

import concourse.bass as bass
import concourse.tile as tile
from concourse import mybir, bacc
import concourse.bass_utils as bass_utils
from concourse.bass_utils import run_bass_kernel_spmd


def _install_axon_profile_hook():
    """Provide antenv.axon_hooks (missing from this image) so
    run_bass_kernel_spmd(trace=True) can capture NTFF profiles via the
    axon PJRT .so, and make artifact upload failures non-fatal."""
    import types
    import ctypes
    import contextlib

    if "antenv.axon_hooks" not in sys.modules:
        mod = types.ModuleType("antenv.axon_hooks")
        _hook_holder = {"hook": None}

        def set_axon_ntff_profile_hook(h):
            _hook_holder["hook"] = h

        def get_axon_ntff_profile_hook():
            return _hook_holder["hook"]

        mod.set_axon_ntff_profile_hook = set_axon_ntff_profile_hook
        mod.get_axon_ntff_profile_hook = get_axon_ntff_profile_hook
        sys.modules["antenv.axon_hooks"] = mod

        so_path = "/opt/axon/libaxon_pjrt.so"
        try:
            lib = ctypes.CDLL(so_path)
            lib.axon_start_nrt_profile.argtypes = [
                ctypes.POINTER(ctypes.c_int64), ctypes.c_size_t]
            lib.axon_start_nrt_profile.restype = ctypes.c_int64
            lib.axon_stop_nrt_profile.argtypes = [ctypes.c_char_p]
            lib.axon_stop_nrt_profile.restype = ctypes.c_int64

            @contextlib.contextmanager
            def _hook(output_dir, device_ids):
                import jax
                jax.devices()
                if device_ids:
                    ids = (ctypes.c_int64 * len(device_ids))(*device_ids)
                    rc = lib.axon_start_nrt_profile(ids, len(device_ids))
                else:
                    rc = lib.axon_start_nrt_profile(None, 0)
                if rc != 0:
                    raise RuntimeError(f"axon_start_nrt_profile rc={rc}")
                try:
                    yield
                finally:
                    n = lib.axon_stop_nrt_profile(str(output_dir).encode())
                    print(f"profile: {n} file(s) written to {output_dir}",
                          file=sys.stderr)

            set_axon_ntff_profile_hook(_hook)
        except OSError:
            pass

    if not getattr(bass_utils.upload_artifacts, "_safe", False):
        _orig_upload = bass_utils.upload_artifacts

        def _safe_upload(tmpdir):
            try:
                return _orig_upload(tmpdir)
            except Exception:
                return str(tmpdir)

        _safe_upload._safe = True
        bass_utils.upload_artifacts = _safe_upload


_install_axon_profile_hook()

F32 = mybir.dt.float32
BF16 = mybir.dt.bfloat16
F8 = mybir.dt.float8e4  # e4m3
AF = mybir.ActivationFunctionType

S = 2048          # sequence length
D = 2048          # d_model
H = 16            # heads
DH = 128          # head dim
NCORES = 8
HPC = H // NCORES  # heads per core = 2
EL = HPC * DH      # local embedding slice = 256
P = 128
QROWS = S // NCORES  # output rows per core = 256
INV_SQRT_DH = float(1.0 / np.sqrt(DH))

CORE_IDS = list(range(NCORES))

_CACHE = {}

# exported for test.py: BassKernelResults of the most recent kernel() call
LAST_RESULTS = None


def _build_module():
    nc = bacc.Bacc("TRN2", target_bir_lowering=False, debug=False,
                   num_devices=NCORES)

    # x stays in [D, S] layout: its fine-grained (1KB-line) DMA pattern
    # interleaves with the AllToAll's internal transfers; one-shot 512KB
    # contiguous x bursts were measured to double the collective durations
    xT_d = nc.dram_tensor("xT", [D, S], BF16, kind="ExternalInput").ap()
    # w*_d[p, dc, e] = W*^T[dc*128 + p, e]
    wq_d = nc.dram_tensor("wq", [P, D // P, EL], BF16,
                          kind="ExternalInput").ap()
    wk_d = nc.dram_tensor("wk", [P, D // P, EL], BF16,
                          kind="ExternalInput").ap()
    wv_d = nc.dram_tensor("wv", [P, D // P, EL], BF16,
                          kind="ExternalInput").ap()
    bq_d = nc.dram_tensor("bq", [P, HPC], F32, kind="ExternalInput").ap()
    bk_d = nc.dram_tensor("bk", [P, HPC], F32, kind="ExternalInput").ap()
    bv_d = nc.dram_tensor("bv", [P, EL], F32, kind="ExternalInput").ap()
    # wo_d[p, ec, f] = Wo^T[ec*128 + p, f]
    wo_d = nc.dram_tensor("wo", [P, H, D], BF16, kind="ExternalInput").ap()
    bo_d = nc.dram_tensor("bo", [1, D], F32, kind="ExternalInput").ap()

    out_d = nc.dram_tensor("out", [QROWS, D], F32, kind="ExternalOutput").ap()

    # per-head collective buffers: [q-shard (dest core), dh, q-within-shard]
    cc_in = [nc.dram_tensor(f"cc_in{h}", [NCORES, P, QROWS], BF16).ap()
             for h in range(HPC)]
    cc_out = [nc.dram_tensor(f"cc_out{h}", [NCORES, P, QROWS], BF16).ap()
              for h in range(HPC)]

    with tile.TileContext(nc, num_cores=NCORES) as tc:
        with (
            tc.tile_pool(name="const", bufs=1) as cpool,
            tc.tile_pool(name="qkv", bufs=1) as qkv_pool,
        ):
            # [128,128] of ones: stationary operand for the replicated-row
            # denominator matmuls
            ones_bf = cpool.tile([P, P], BF16, name="ones_bf")
            nc.vector.memset(ones_bf[:], 1.0)

            # 4 distinct diagonal-block causal masks (bf16 0/1).
            # Block (qb, kc) with off = kc*128 - qb*512 keeps q >= k + off;
            # off only takes values {0,128,256,384} (i = kc - 4*qb).
            masks = cpool.tile([P, 4, 512], BF16, name="masks")
            nc.vector.memset(masks[:], 1.0)
            for i in range(4):
                nc.gpsimd.affine_select(
                    out=masks[:, i, :], in_=masks[:, i, :],
                    compare_op=mybir.AluOpType.is_ge,
                    fill=0.0, base=-(i * P), channel_multiplier=-1,
                    pattern=[[1, 512]])

            # per-head Q^T/K^T [dh, s] (bf16, Q pre-scaled by 1/sqrt(dh)) and
            # V [s, e_local] (bf16) resident in SBUF
            QT = [qkv_pool.tile([P, S], BF16, name=f"QT{h}") for h in range(HPC)]
            KT = [qkv_pool.tile([P, S], BF16, name=f"KT{h}") for h in range(HPC)]
            V_t = qkv_pool.tile([P, S // P, EL], BF16, name="V_t")

            # output-projection weights pool; loaded chunk-wise during
            # phase 1 on the Sync queue (DMA bandwidth has headroom there)
            p3 = tc.alloc_tile_pool(name="p3", bufs=1)
            wo_t = p3.tile([P, H, D], BF16, name="wo_t")
            bo_t = p3.tile([P, D], F32, name="bo_t")

            wpool = tc.alloc_tile_pool(name="w", bufs=1)
            bq_t = wpool.tile([P, HPC], F32, name="bq_t")
            bk_t = wpool.tile([P, HPC], F32, name="bk_t")
            bv_t = wpool.tile([P, EL], F32, name="bv_t")
            # full-width (both heads) weight tiles, loaded once in pass 0
            wq_t = wpool.tile([P, D // P, EL], BF16, name="wq_t")
            wk_t = wpool.tile([P, D // P, EL], BF16, name="wk_t")
            wv_t = wpool.tile([P, D // P, EL], BF16, name="wv_t")

            xt_pool = tc.alloc_tile_pool(name="xt", bufs=4)
            # pass-1 x tiles prefetched during pass-0 attention (keyed by
            # (sbi, dc4)) so the Sync queue isn't blocked behind cc stores
            prefetched = {}

            def prefetch_xt(sbi, dc4):
                xt = xt_pool.tile([P, 4, 512], BF16, name="xt")
                nc.sync.dma_start(
                    xt[:],
                    xT_d[dc4 * 4 * P:(dc4 + 1) * 4 * P,
                         sbi * 512:(sbi + 1) * 512]
                    .rearrange("(i p) s -> p i s", p=P))
                return xt

            def qkv_pass(h):
                """Q/K projection for head h; pass 0 also computes V for
                both heads (keeps the LDWEIGHTS pipe under the matmul time
                in both passes)."""
                with (
                    tc.tile_pool(name=f"ps_qk{h}", bufs=2, space="PSUM") as ps_qk,
                    tc.tile_pool(name=f"ps_v{h}", bufs=1, space="PSUM") as ps_v,
                ):
                    e_sl = slice(h * DH, (h + 1) * DH)

                    def load_w_chunk(c4, engs):
                        dsl = slice(c4 * (D // P // 4), (c4 + 1) * (D // P // 4))
                        engs[0].dma_start(wq_t[:, dsl, :], wq_d[:, dsl, :])
                        engs[1].dma_start(wk_t[:, dsl, :], wk_d[:, dsl, :])
                        engs[2].dma_start(wv_t[:, dsl, :], wv_d[:, dsl, :])

                    if h == 0:
                        # first x tile ahead of everything on Sync; weight
                        # chunk 0 right behind it, remaining chunks
                        # interleaved with the x stream below; biases on the
                        # otherwise-idle GpSimd queue
                        # split the very first tiles so the first matmul
                        # waits on ~192KB, not ~1.3MB
                        xt0 = xt_pool.tile([P, 4, 512], BF16, name="xt")
                        nc.sync.dma_start(
                            xt0[:, 0:1, :],
                            xT_d[0:P, 0:512]
                            .rearrange("(i p) s -> p i s", p=P))
                        nc.scalar.dma_start(wq_t[:, 0:1, :], wq_d[:, 0:1, :])
                        nc.scalar.dma_start(wk_t[:, 0:1, :], wk_d[:, 0:1, :])
                        nc.gpsimd.dma_start(wv_t[:, 0:1, :], wv_d[:, 0:1, :])
                        nc.sync.dma_start(
                            xt0[:, 1:4, :],
                            xT_d[P:4 * P, 0:512]
                            .rearrange("(i p) s -> p i s", p=P))
                        dsl0 = slice(1, D // P // 4)
                        nc.sync.dma_start(wq_t[:, dsl0, :], wq_d[:, dsl0, :])
                        nc.scalar.dma_start(wk_t[:, dsl0, :], wk_d[:, dsl0, :])
                        nc.gpsimd.dma_start(wv_t[:, dsl0, :], wv_d[:, dsl0, :])
                        # chunk 1 upfront on the idle scalar/gpsimd queues;
                        # chunks 2-3 are deferred past the startup bandwidth
                        # crunch (WAW-pinned below to the sbi-0 activations —
                        # dep-free DMAs get hoisted to t=0 by the scheduler)
                        load_w_chunk(1, [nc.scalar, nc.gpsimd, nc.scalar])
                        nc.gpsimd.dma_start(bq_t[:], bq_d[:])
                        nc.gpsimd.dma_start(bk_t[:], bk_d[:])
                        nc.gpsimd.dma_start(bv_t[:], bv_d[:])
                        bo_row = wpool.tile([1, D], F32, name="bo_row")
                        nc.gpsimd.dma_start(bo_row[:], bo_d[:])
                        # broadcast to all partitions once, during pass 0
                        # (gpsimd is idle and no collective is in flight)
                        nc.gpsimd.partition_broadcast(bo_t[:], bo_row[:])

                    for sbi in range(S // 512):
                        qp = ps_qk.tile([P, 512], F32, name="qp")
                        kp = ps_qk.tile([P, 512], F32, name="kp")
                        # one PSUM bank per accumulation group (start=True
                        # clears the whole bank)
                        v_ps_tiles = ([ps_v.tile([P, EL], F32, name=f"v_ps{j}")
                                       for j in range(4)] if h == 0 else None)
                        for dc4 in range(D // P // 4):
                            if h == 0 and sbi == 0 and dc4 == 0:
                                xt = xt0
                            elif (sbi, dc4) in prefetched:
                                xt = prefetched.pop((sbi, dc4))
                            else:
                                xt = prefetch_xt(sbi, dc4)
                            for i in range(4):
                                dc = dc4 * 4 + i
                                st, sp = dc == 0, dc == (D // P - 1)
                                xti = xt[:, i, :]
                                nc.tensor.matmul(qp[:], wq_t[:, dc, e_sl], xti,
                                                 start=st, stop=sp)
                                nc.tensor.matmul(kp[:], wk_t[:, dc, e_sl], xti,
                                                 start=st, stop=sp)
                                if h == 0:
                                    for j in range(4):
                                        nc.tensor.matmul(
                                            v_ps_tiles[j][:],
                                            xt[:, i, j * P:(j + 1) * P],
                                            wv_t[:, dc, :], start=st, stop=sp)
                        s_sl = slice(sbi * 512, (sbi + 1) * 512)
                        nc.scalar.activation(QT[h][:, s_sl], qp[:], AF.Identity,
                                             bias=bq_t[:, h:h + 1],
                                             scale=INV_SQRT_DH)
                        nc.scalar.activation(KT[h][:, s_sl], kp[:], AF.Identity,
                                             bias=bk_t[:, h:h + 1])
                        if h == 0 and sbi == 0:
                            # release the deferred weight chunks now: dummy
                            # 1-element writes depending on QT pin the DMAs
                            # (WAW) past the startup window
                            for wt, wd in ((wq_t, wq_d), (wk_t, wk_d),
                                           (wv_t, wv_d)):
                                nc.vector.tensor_add(
                                    wt[0:1, 8, 0:1],
                                    QT[0][0:1, 0:1], QT[0][0:1, 0:1])
                                dsl23 = slice(8, D // P)
                                nc.sync.dma_start(wt[:, dsl23, :],
                                                  wd[:, dsl23, :])
                        if h == 0:
                            for j in range(4):
                                nc.vector.tensor_add(V_t[:, sbi * 4 + j, :],
                                                     v_ps_tiles[j][:], bv_t[:])
                        # spread the back half of the Wo prefetch across
                        # pass 1 (the front half goes out during attention 0;
                        # nothing at startup — Wo isn't needed until phase 3,
                        # and early DMA bandwidth is the startup bottleneck)
                        if h == 1:
                            wi = 4 + sbi
                            nc.sync.dma_start(
                                wo_t[:, wi * 2:(wi + 1) * 2, :],
                                wo_d[:, wi * 2:(wi + 1) * 2, :])

            def attention(h):
                """Causal attention for head h, software-pipelined at
                chunk-pair granularity: score pairs are issued 2 pairs ahead
                of the AV/den matmuls (the pair issue precedes the previous
                pair's AV block, giving ~2.6us of PE cover vs the ~1.1us
                exp-pair latency). Pair-sized exps amortize the scalar
                engine's fixed per-instruction overhead, keeping it well
                under the PE."""
                with (
                    tc.tile_pool(name=f"pt{h}", bufs=8) as pt_pool,
                    tc.tile_pool(name=f"att_sb{h}", bufs=2) as att_sb,
                    tc.tile_pool(name=f"ps_s{h}", bufs=2, space="PSUM") as ps_s,
                    tc.tile_pool(name=f"ps_at{h}", bufs=2, space="PSUM") as ps_at,
                    tc.tile_pool(name=f"ps_den{h}", bufs=2, space="PSUM") as ps_den,
                ):
                    # The softmax denominator is accumulated over k-chunks on
                    # the vector engine in bf16 (den_bf), then reduced across
                    # partitions by ONE replicated-ones matmul per qb — 4 PE
                    # matmuls/head instead of 40. Eight pt buffers keep the
                    # (lagging, serial) vector den chain from ever blocking
                    # the exp pipeline, and the normalize of qb runs deferred
                    # under qb+1's prologue so the PE never waits on it.
                    pending = [None]

                    def flush_norm():
                        if pending[0] is None:
                            return
                        qb0, at0, den0 = pending[0]
                        pending[0] = None
                        den_ps = ps_den.tile([P, 512], F32, name="den_ps")
                        nc.tensor.matmul(den_ps[:], ones_bf[:], den0[:],
                                         start=True, stop=True)
                        rd = att_sb.tile([P, 512], F32, name="rd")
                        nc.vector.reciprocal_approx_fast(out=rd[:],
                                                         in_=den_ps[:])
                        at_bf = att_sb.tile([P, 512], BF16, name="at_bf")
                        nc.vector.tensor_mul(at_bf[:], at0[:], rd[:])
                        for i in range(2):
                            nc.sync.dma_start(
                                cc_in[h][2 * qb0 + i, :, :],
                                at_bf[:, i * QROWS:(i + 1) * QROWS])
                        if h == 0:
                            # front half of the Wo prefetch, behind the
                            # cc stores on the idle Sync queue
                            nc.sync.dma_start(
                                wo_t[:, qb0 * 2:(qb0 + 1) * 2, :],
                                wo_d[:, qb0 * 2:(qb0 + 1) * 2, :])

                    for qb in range(S // 512):
                        nkc = 4 * (qb + 1)  # causal: only k <= q blocks
                        npair = nkc // 2
                        q_sl = slice(qb * 512, (qb + 1) * 512)
                        at_ps = ps_at.tile([P, 512], F32, name="at_ps")
                        den_bf = att_sb.tile([P, 512], BF16, name="den_bf")
                        pts = [None] * npair

                        def issue_pair(pr):
                            s_ps = ps_s.tile([P, 2, 512], F32, name="s_ps")
                            for u in range(2):
                                kc = 2 * pr + u
                                nc.tensor.matmul(
                                    s_ps[:, u, :],
                                    KT[h][:, kc * P:(kc + 1) * P],
                                    QT[h][:, q_sl], start=True, stop=True)
                            pt = pt_pool.tile([P, 2, 512], BF16, name="pt")
                            nc.scalar.activation(pt[:], s_ps[:], AF.Exp)
                            if pr >= 2 * qb:
                                # diagonal pair: zero the strictly-upper parts
                                i0 = 2 * (pr - 2 * qb)
                                nc.vector.tensor_mul(
                                    pt[:], pt[:], masks[:, i0:i0 + 2, :])
                            # denominator chunk partial sums (vector, bf16)
                            if pr == 0:
                                nc.vector.tensor_add(den_bf[:], pt[:, 0, :],
                                                     pt[:, 1, :])
                            else:
                                for u in range(2):
                                    nc.vector.tensor_add(
                                        den_bf[:], den_bf[:], pt[:, u, :])
                            pts[pr] = pt

                        for pr in range(min(2, npair)):
                            issue_pair(pr)
                        # previous qb's normalize runs here, off the PE's
                        # critical path (its den chain finished long ago)
                        flush_norm()
                        for pr in range(npair):
                            if pr + 2 < npair:
                                issue_pair(pr + 2)
                            for u in range(2):
                                kc = 2 * pr + u
                                st, sp = kc == 0, kc == nkc - 1
                                nc.tensor.matmul(
                                    at_ps[:], V_t[:, kc, h * DH:(h + 1) * DH],
                                    pts[pr][:, u, :], start=st, stop=sp)
                        pending[0] = (qb, at_ps, den_bf)
                    flush_norm()
                # redistribute this head's outputs; the next head's QKV
                # pass + attention run under this collective
                nc.gpsimd.collective_compute(
                    "AllToAll", mybir.AluOpType.bypass,
                    replica_groups=[CORE_IDS],
                    ins=[cc_in[h][:]], outs=[cc_out[h][:]])

            for h in range(HPC):
                qkv_pass(h)
                if h == 0:
                    # prefetch pass-1's first x tiles now so they land on the
                    # Sync queue ahead of attention-0's cc stores
                    for sbi, dc4 in [(0, 0), (0, 1), (0, 2), (0, 3)]:
                        prefetched[(sbi, dc4)] = prefetch_xt(sbi, dc4)
                attention(h)

            # ---------------- Phase 3: output projection ----------------
            with (
                tc.tile_pool(name="osb", bufs=3) as osb,
                tc.tile_pool(name="ps_o", bufs=1, space="PSUM") as ps_o,
            ):
                # cc_out[h][j, p, q] = attn^T for global head (2j+h), own q
                # slice. Two tiles per head (j halves) so the j<4 matmuls can
                # start while the second half is still loading.
                # aT loads stay on Sync only: putting them on the scalar
                # queue head-of-line blocks attention work behind the
                # collective wait, and any gpsimd (software-DGE) DMA in
                # flight during an AllToAll roughly doubles its duration
                # (descgen contention with NRT's collective processing).
                NH = NCORES // 2
                aT = [[p3.tile([P, NH, QROWS], BF16, name=f"aT{h}_{half}")
                       for half in range(2)] for h in range(HPC)]
                for h in range(HPC):
                    for half in range(2):
                        if h == 1:
                            # h1 is on the critical path after AllToAll#1:
                            # quarter-granularity loads let the first
                            # output-projection matmuls start ~1.5us sooner
                            for q4 in range(2):
                                j0 = half * NH + q4 * (NH // 2)
                                nc.sync.dma_start(
                                    aT[h][half][:, q4 * (NH // 2):
                                                 (q4 + 1) * (NH // 2), :],
                                    cc_out[h][j0:j0 + NH // 2]
                                    .rearrange("j p q -> p j q"))
                        else:
                            nc.sync.dma_start(
                                aT[h][half][:],
                                cc_out[h][half * NH:(half + 1) * NH]
                                .rearrange("j p q -> p j q"))

                # all 8 (qc, fb) groups live in 8 PSUM banks at once; all
                # head-0 contributions (available after the first AllToAll)
                # run first, overlapping the second AllToAll
                blocks = [(qc, fb) for qc in range(QROWS // P)
                          for fb in range(D // 512)]
                o_ps = {b: ps_o.tile([P, 512], F32, name=f"o_ps_{b[0]}_{b[1]}")
                        for b in blocks}
                for h in range(HPC):
                    for half in range(2):
                        for qc, fb in blocks:
                            for jj in range(NH):
                                j = half * NH + jj
                                nc.tensor.matmul(
                                    o_ps[(qc, fb)][:],
                                    aT[h][half][:, jj, qc * P:(qc + 1) * P],
                                    wo_t[:, 2 * j + h,
                                         fb * 512:(fb + 1) * 512],
                                    start=(h == 0 and j == 0),
                                    stop=(h == HPC - 1 and j == NCORES - 1))
                for gi, (qc, fb) in enumerate(blocks):
                    o_sb = osb.tile([P, 512], F32, name="o_sb")
                    nc.vector.tensor_add(o_sb[:], o_ps[(qc, fb)][:],
                                         bo_t[:, fb * 512:(fb + 1) * 512])
                    # alternate store queues so the 2MB of output doesn't
                    # serialize on one DMA ring at the very end
                    eng = nc.sync if gi % 2 == 0 else nc.scalar
                    eng.dma_start(
                        out_d[qc * P:(qc + 1) * P, fb * 512:(fb + 1) * 512],
                        o_sb[:])
            xt_pool.release()
            wpool.release()
            p3.release()

    nc.finalize()
    return nc


def kernel(x, mask, Wq, bq, Wk, bk, Wv, bv, Wo, bo):
    """Full-input MHA forward. Returns the full (2048, 2048) fp32 output.

    The mask input is assumed to be the strictly-upper-triangular causal mask
    the reference generates; causality is applied structurally on-device.
    """
    global LAST_RESULTS
    if "nc" not in _CACHE:
        _CACHE["nc"] = _build_module()
    nc = _CACHE["nc"]

    x = np.asarray(x, dtype=np.float32)
    Wq = np.asarray(Wq, dtype=np.float32)
    Wk = np.asarray(Wk, dtype=np.float32)
    Wv = np.asarray(Wv, dtype=np.float32)
    Wo = np.asarray(Wo, dtype=np.float32)
    bq = np.asarray(bq, dtype=np.float32)
    bk = np.asarray(bk, dtype=np.float32)
    bv = np.asarray(bv, dtype=np.float32)
    bo = np.asarray(bo, dtype=np.float32)

    bf = ml_dtypes.bfloat16
    xT = np.ascontiguousarray(x.T).astype(bf)
    # woT[p, ec, f] = Wo^T[ec*128 + p, f]
    woT_bf = np.ascontiguousarray(
        Wo.T.astype(bf).reshape(H, P, D).transpose(1, 0, 2))
    bo_row_h = np.ascontiguousarray(bo.reshape(1, D))

    def tile_w(W):  # [EL, D] slice -> [P, D//P, EL] with w[p, dc, e]
        return np.ascontiguousarray(
            W.T.astype(bf).reshape(D // P, P, EL).transpose(1, 0, 2))

    in_maps = []
    for c in range(NCORES):
        e_sl = slice(c * EL, (c + 1) * EL)
        in_maps.append({
            "xT": xT,
            "wq": tile_w(Wq[e_sl, :]),
            "wk": tile_w(Wk[e_sl, :]),
            "wv": tile_w(Wv[e_sl, :]),
            # bias layout [dh, head]; Q bias pre-scaled by 1/sqrt(dh)
            "bq": np.ascontiguousarray((bq[e_sl] * INV_SQRT_DH).reshape(HPC, P).T),
            "bk": np.ascontiguousarray(bk[e_sl].reshape(HPC, P).T),
            "bv": np.ascontiguousarray(np.broadcast_to(bv[e_sl], (P, EL))),
            "wo": woT_bf,
            "bo": bo_row_h,
        })

    res = run_bass_kernel_spmd(nc, in_maps, CORE_IDS)
    LAST_RESULTS = res
    return np.concatenate([res.results[c]["out"] for c in range(NCORES)], axis=0)


# revision 77
# speedup vs baseline: 1.0891x; 1.0891x over previous
"""Multi-head attention (16 heads, d_model=2048, seq=2048, causal) on 8 trn2 cores.

Sharding: tensor-parallel over heads (2 heads/core) for QKV projection and
attention; two per-head AllToAlls redistribute the (normalized) per-head
attention outputs so each core holds all heads for a 256-row query slice;
each core then runs the full output projection for its slice and the host
concatenates the 8 slices.

Structure (v3):
 - Phase 1 and attention are interleaved per head: pass 0 computes Q/K for
   head 0 plus V for both heads, then runs head-0 attention and triggers its
   AllToAll at ~45% of the kernel; pass 1 (Q/K for head 1) and head-1
   attention execute under that collective, hiding its rendezvous skew and
   transfer time entirely.
 - Attention is software-pipelined: score matmuls run 2 chunk-pairs ahead of
   the AV/denominator matmuls; exp is done on [128,2,512] pairs to amortize
   the scalar engine's fixed per-instruction overhead (scalar stays below PE).
 - Softmax denominator via a replicated-ones [128x128] stationary matmul into
   a [128,512] PSUM tile; normalization is a full-lane reciprocal_approx_fast
   + tensor_mul on the vector engine.
 - Causal mask applied structurally: strictly-upper 128x512 blocks skipped;
   the 4 distinct diagonal-block patterns are precomputed bf16 0/1 masks
   applied with vector tensor_mul (gpsimd stays free so AllToAlls trigger
   immediately).
 - All matmul operands bf16 (fp32 PSUM accumulation).
"""
import sys

sys.path.insert(0, "/opt/trn_rl_repo")

import numpy as np
import ml_dtypes

import concourse.bass as bass
import concourse.tile as tile
from concourse import mybir, bacc
import concourse.bass_utils as bass_utils
from concourse.bass_utils import run_bass_kernel_spmd


def _install_axon_profile_hook():
    """Provide antenv.axon_hooks (missing from this image) so
    run_bass_kernel_spmd(trace=True) can capture NTFF profiles via the
    axon PJRT .so, and make artifact upload failures non-fatal."""
    import types
    import ctypes
    import contextlib

    if "antenv.axon_hooks" not in sys.modules:
        mod = types.ModuleType("antenv.axon_hooks")
        _hook_holder = {"hook": None}

        def set_axon_ntff_profile_hook(h):
            _hook_holder["hook"] = h

        def get_axon_ntff_profile_hook():
            return _hook_holder["hook"]

        mod.set_axon_ntff_profile_hook = set_axon_ntff_profile_hook
        mod.get_axon_ntff_profile_hook = get_axon_ntff_profile_hook
        sys.modules["antenv.axon_hooks"] = mod

        so_path = "/opt/axon/libaxon_pjrt.so"
        try:
            lib = ctypes.CDLL(so_path)
            lib.axon_start_nrt_profile.argtypes = [
                ctypes.POINTER(ctypes.c_int64), ctypes.c_size_t]
            lib.axon_start_nrt_profile.restype = ctypes.c_int64
            lib.axon_stop_nrt_profile.argtypes = [ctypes.c_char_p]
            lib.axon_stop_nrt_profile.restype = ctypes.c_int64

            @contextlib.contextmanager
            def _hook(output_dir, device_ids):
                import jax
                jax.devices()
                if device_ids:
                    ids = (ctypes.c_int64 * len(device_ids))(*device_ids)
                    rc = lib.axon_start_nrt_profile(ids, len(device_ids))
                else:
                    rc = lib.axon_start_nrt_profile(None, 0)
                if rc != 0:
                    raise RuntimeError(f"axon_start_nrt_profile rc={rc}")
                try:
                    yield
                finally:
                    n = lib.axon_stop_nrt_profile(str(output_dir).encode())
                    print(f"profile: {n} file(s) written to {output_dir}",
                          file=sys.stderr)

            set_axon_ntff_profile_hook(_hook)
        except OSError:
            pass

    if not getattr(bass_utils.upload_artifacts, "_safe", False):
        _orig_upload = bass_utils.upload_artifacts

        def _safe_upload(tmpdir):
            try:
                return _orig_upload(tmpdir)
            except Exception:
                return str(tmpdir)

        _safe_upload._safe = True
        bass_utils.upload_artifacts = _safe_upload


_install_axon_profile_hook()

F32 = mybir.dt.float32
BF16 = mybir.dt.bfloat16
F8 = mybir.dt.float8e4  # e4m3
AF = mybir.ActivationFunctionType

S = 2048          # sequence length
D = 2048          # d_model
H = 16            # heads
DH = 128          # head dim
NCORES = 8
HPC = H // NCORES  # heads per core = 2
EL = HPC * DH      # local embedding slice = 256
P = 128
QROWS = S // NCORES  # output rows per core = 256
INV_SQRT_DH = float(1.0 / np.sqrt(DH))

CORE_IDS = list(range(NCORES))

_CACHE = {}

# exported for test.py: BassKernelResults of the most recent kernel() call
LAST_RESULTS = None


def _build_module():
    nc = bacc.Bacc("TRN2", target_bir_lowering=False, debug=False,
                   num_devices=NCORES)

    # x stays in [D, S] layout: its fine-grained (1KB-line) DMA pattern
    # interleaves with the AllToAll's internal transfers; one-shot 512KB
    # contiguous x bursts were measured to double the collective durations
    xT_d = nc.dram_tensor("xT", [D, S], BF16, kind="ExternalInput").ap()
    # w*_d[p, dc, e] = W*^T[dc*128 + p, e]
    wq_d = nc.dram_tensor("wq", [P, D // P, EL], BF16,
                          kind="ExternalInput").ap()
    wk_d = nc.dram_tensor("wk", [P, D // P, EL], BF16,
                          kind="ExternalInput").ap()
    wv_d = nc.dram_tensor("wv", [P, D // P, EL], BF16,
                          kind="ExternalInput").ap()
    bq_d = nc.dram_tensor("bq", [P, HPC], F32, kind="ExternalInput").ap()
    bk_d = nc.dram_tensor("bk", [P, HPC], F32, kind="ExternalInput").ap()
    bv_d = nc.dram_tensor("bv", [P, EL], F32, kind="ExternalInput").ap()
    # wo_d[p, ec, f] = Wo^T[ec*128 + p, f]
    wo_d = nc.dram_tensor("wo", [P, H, D], BF16, kind="ExternalInput").ap()
    bo_d = nc.dram_tensor("bo", [1, D], F32, kind="ExternalInput").ap()

    out_d = nc.dram_tensor("out", [QROWS, D], F32, kind="ExternalOutput").ap()

    # per-head collective buffers: [q-shard (dest core), dh, q-within-shard]
    cc_in = [nc.dram_tensor(f"cc_in{h}", [NCORES, P, QROWS], BF16).ap()
             for h in range(HPC)]
    cc_out = [nc.dram_tensor(f"cc_out{h}", [NCORES, P, QROWS], BF16).ap()
              for h in range(HPC)]

    with tile.TileContext(nc, num_cores=NCORES) as tc:
        with (
            tc.tile_pool(name="const", bufs=1) as cpool,
            tc.tile_pool(name="qkv", bufs=1) as qkv_pool,
        ):
            # [128,128] of ones: stationary operand for the replicated-row
            # denominator matmuls
            ones_bf = cpool.tile([P, P], BF16, name="ones_bf")
            nc.vector.memset(ones_bf[:], 1.0)

            # 4 distinct diagonal-block causal masks (bf16 0/1).
            # Block (qb, kc) with off = kc*128 - qb*512 keeps q >= k + off;
            # off only takes values {0,128,256,384} (i = kc - 4*qb).
            masks = cpool.tile([P, 4, 512], BF16, name="masks")
            nc.vector.memset(masks[:], 1.0)
            for i in range(4):
                nc.gpsimd.affine_select(
                    out=masks[:, i, :], in_=masks[:, i, :],
                    compare_op=mybir.AluOpType.is_ge,
                    fill=0.0, base=-(i * P), channel_multiplier=-1,
                    pattern=[[1, 512]])

            # per-head Q^T/K^T [dh, s] (bf16, Q pre-scaled by 1/sqrt(dh)) and
            # V [s, e_local] (bf16) resident in SBUF
            QT = [qkv_pool.tile([P, S], BF16, name=f"QT{h}") for h in range(HPC)]
            KT = [qkv_pool.tile([P, S], BF16, name=f"KT{h}") for h in range(HPC)]
            V_t = qkv_pool.tile([P, S // P, EL], BF16, name="V_t")

            # output-projection weights pool; loaded chunk-wise during
            # phase 1 on the Sync queue (DMA bandwidth has headroom there)
            p3 = tc.alloc_tile_pool(name="p3", bufs=1)
            wo_t = p3.tile([P, H, D], BF16, name="wo_t")
            bo_t = p3.tile([P, D], F32, name="bo_t")

            wpool = tc.alloc_tile_pool(name="w", bufs=1)
            bq_t = wpool.tile([P, HPC], F32, name="bq_t")
            bk_t = wpool.tile([P, HPC], F32, name="bk_t")
            bv_t = wpool.tile([P, EL], F32, name="bv_t")
            # full-width (both heads) weight tiles, loaded once in pass 0
            wq_t = wpool.tile([P, D // P, EL], BF16, name="wq_t")
            wk_t = wpool.tile([P, D // P, EL], BF16, name="wk_t")
            wv_t = wpool.tile([P, D // P, EL], BF16, name="wv_t")

            xt_pool = tc.alloc_tile_pool(name="xt", bufs=4)
            # pass-1 x tiles prefetched during pass-0 attention (keyed by
            # (sbi, dc4)) so the Sync queue isn't blocked behind cc stores
            prefetched = {}

            def prefetch_xt(sbi, dc4):
                xt = xt_pool.tile([P, 4, 512], BF16, name="xt")
                nc.sync.dma_start(
                    xt[:],
                    xT_d[dc4 * 4 * P:(dc4 + 1) * 4 * P,
                         sbi * 512:(sbi + 1) * 512]
                    .rearrange("(i p) s -> p i s", p=P))
                return xt

            def qkv_pass(h):
                """Q/K projection for head h; pass 0 also computes V for
                both heads (keeps the LDWEIGHTS pipe under the matmul time
                in both passes)."""
                with (
                    tc.tile_pool(name=f"ps_qk{h}", bufs=2, space="PSUM") as ps_qk,
                    tc.tile_pool(name=f"ps_v{h}", bufs=1, space="PSUM") as ps_v,
                ):
                    e_sl = slice(h * DH, (h + 1) * DH)

                    def load_w_chunk(c4, engs):
                        dsl = slice(c4 * (D // P // 4), (c4 + 1) * (D // P // 4))
                        engs[0].dma_start(wq_t[:, dsl, :], wq_d[:, dsl, :])
                        engs[1].dma_start(wk_t[:, dsl, :], wk_d[:, dsl, :])
                        engs[2].dma_start(wv_t[:, dsl, :], wv_d[:, dsl, :])

                    if h == 0:
                        # first x tile ahead of everything on Sync; weight
                        # chunk 0 right behind it, remaining chunks
                        # interleaved with the x stream below; biases on the
                        # otherwise-idle GpSimd queue
                        # split the very first tiles so the first matmul
                        # waits on ~192KB, not ~1.3MB
                        xt0 = xt_pool.tile([P, 4, 512], BF16, name="xt")
                        nc.sync.dma_start(
                            xt0[:, 0:1, :],
                            xT_d[0:P, 0:512]
                            .rearrange("(i p) s -> p i s", p=P))
                        nc.scalar.dma_start(wq_t[:, 0:1, :], wq_d[:, 0:1, :])
                        nc.scalar.dma_start(wk_t[:, 0:1, :], wk_d[:, 0:1, :])
                        nc.gpsimd.dma_start(wv_t[:, 0:1, :], wv_d[:, 0:1, :])
                        nc.sync.dma_start(
                            xt0[:, 1:4, :],
                            xT_d[P:4 * P, 0:512]
                            .rearrange("(i p) s -> p i s", p=P))
                        dsl0 = slice(1, D // P // 4)
                        nc.sync.dma_start(wq_t[:, dsl0, :], wq_d[:, dsl0, :])
                        nc.scalar.dma_start(wk_t[:, dsl0, :], wk_d[:, dsl0, :])
                        nc.gpsimd.dma_start(wv_t[:, dsl0, :], wv_d[:, dsl0, :])
                        # chunk 1 upfront on the idle scalar/gpsimd queues;
                        # chunks 2-3 are deferred past the startup bandwidth
                        # crunch (WAW-pinned below to the sbi-0 activations —
                        # dep-free DMAs get hoisted to t=0 by the scheduler)
                        load_w_chunk(1, [nc.scalar, nc.gpsimd, nc.scalar])
                        nc.gpsimd.dma_start(bq_t[:], bq_d[:])
                        nc.gpsimd.dma_start(bk_t[:], bk_d[:])
                        nc.gpsimd.dma_start(bv_t[:], bv_d[:])
                        bo_row = wpool.tile([1, D], F32, name="bo_row")
                        nc.gpsimd.dma_start(bo_row[:], bo_d[:])
                        # broadcast to all partitions once, during pass 0
                        # (gpsimd is idle and no collective is in flight)
                        nc.gpsimd.partition_broadcast(bo_t[:], bo_row[:])

                    for sbi in range(S // 512):
                        qp = ps_qk.tile([P, 512], F32, name="qp")
                        kp = ps_qk.tile([P, 512], F32, name="kp")
                        # one PSUM bank per accumulation group (start=True
                        # clears the whole bank)
                        v_ps_tiles = ([ps_v.tile([P, EL], F32, name=f"v_ps{j}")
                                       for j in range(4)] if h == 0 else None)
                        for dc4 in range(D // P // 4):
                            if h == 0 and sbi == 0 and dc4 == 0:
                                xt = xt0
                            elif (sbi, dc4) in prefetched:
                                xt = prefetched.pop((sbi, dc4))
                            else:
                                xt = prefetch_xt(sbi, dc4)
                            for i in range(4):
                                dc = dc4 * 4 + i
                                st, sp = dc == 0, dc == (D // P - 1)
                                xti = xt[:, i, :]
                                nc.tensor.matmul(qp[:], wq_t[:, dc, e_sl], xti,
                                                 start=st, stop=sp)
                                nc.tensor.matmul(kp[:], wk_t[:, dc, e_sl], xti,
                                                 start=st, stop=sp)
                                if h == 0:
                                    for j in range(4):
                                        nc.tensor.matmul(
                                            v_ps_tiles[j][:],
                                            xt[:, i, j * P:(j + 1) * P],
                                            wv_t[:, dc, :], start=st, stop=sp)
                        s_sl = slice(sbi * 512, (sbi + 1) * 512)
                        nc.scalar.activation(QT[h][:, s_sl], qp[:], AF.Identity,
                                             bias=bq_t[:, h:h + 1],
                                             scale=INV_SQRT_DH)
                        nc.scalar.activation(KT[h][:, s_sl], kp[:], AF.Identity,
                                             bias=bk_t[:, h:h + 1])
                        if h == 0 and sbi == 0:
                            # release the deferred weight chunks now: dummy
                            # 1-element writes depending on QT pin the DMAs
                            # (WAW) past the startup window
                            for wt, wd in ((wq_t, wq_d), (wk_t, wk_d),
                                           (wv_t, wv_d)):
                                nc.vector.tensor_add(
                                    wt[0:1, 8, 0:1],
                                    QT[0][0:1, 0:1], QT[0][0:1, 0:1])
                                dsl23 = slice(8, D // P)
                                nc.sync.dma_start(wt[:, dsl23, :],
                                                  wd[:, dsl23, :])
                        if h == 0:
                            for j in range(4):
                                nc.vector.tensor_add(V_t[:, sbi * 4 + j, :],
                                                     v_ps_tiles[j][:], bv_t[:])
                        # spread the back half of the Wo prefetch across
                        # pass 1 (the front half goes out during attention 0;
                        # nothing at startup — Wo isn't needed until phase 3,
                        # and early DMA bandwidth is the startup bottleneck)
                        if h == 1:
                            wi = 4 + sbi
                            nc.sync.dma_start(
                                wo_t[:, wi * 2:(wi + 1) * 2, :],
                                wo_d[:, wi * 2:(wi + 1) * 2, :])

            def attention(h):
                """Causal attention for head h, software-pipelined at
                chunk-pair granularity: score pairs are issued 2 pairs ahead
                of the AV/den matmuls (the pair issue precedes the previous
                pair's AV block, giving ~2.6us of PE cover vs the ~1.1us
                exp-pair latency). Pair-sized exps amortize the scalar
                engine's fixed per-instruction overhead, keeping it well
                under the PE."""
                with (
                    tc.tile_pool(name=f"pt{h}", bufs=8) as pt_pool,
                    tc.tile_pool(name=f"att_sb{h}", bufs=2) as att_sb,
                    tc.tile_pool(name=f"ps_s{h}", bufs=2, space="PSUM") as ps_s,
                    tc.tile_pool(name=f"ps_at{h}", bufs=2, space="PSUM") as ps_at,
                    tc.tile_pool(name=f"ps_den{h}", bufs=2, space="PSUM") as ps_den,
                ):
                    # The softmax denominator is accumulated over k-chunks on
                    # the vector engine in bf16 (den_bf), then reduced across
                    # partitions by ONE replicated-ones matmul per qb — 4 PE
                    # matmuls/head instead of 40. Eight pt buffers keep the
                    # (lagging, serial) vector den chain from ever blocking
                    # the exp pipeline, and the normalize of qb runs deferred
                    # under qb+1's prologue so the PE never waits on it.
                    pending = [None]

                    def flush_norm():
                        if pending[0] is None:
                            return
                        qb0, at0, den0 = pending[0]
                        pending[0] = None
                        # combine the two half-accumulators, then reduce
                        # across partitions with one replicated-ones matmul
                        den_c = att_sb.tile([P, 512], BF16, name="den_c")
                        nc.vector.tensor_add(den_c[:], den0[:, 0, :],
                                             den0[:, 1, :])
                        den_ps = ps_den.tile([P, 512], F32, name="den_ps")
                        nc.tensor.matmul(den_ps[:], ones_bf[:], den_c[:],
                                         start=True, stop=True)
                        rd = att_sb.tile([P, 512], F32, name="rd")
                        nc.vector.reciprocal_approx_fast(out=rd[:],
                                                         in_=den_ps[:])
                        at_bf = att_sb.tile([P, 512], BF16, name="at_bf")
                        nc.vector.tensor_mul(at_bf[:], at0[:], rd[:])
                        for i in range(2):
                            nc.sync.dma_start(
                                cc_in[h][2 * qb0 + i, :, :],
                                at_bf[:, i * QROWS:(i + 1) * QROWS])
                        if h == 0:
                            # front half of the Wo prefetch, behind the
                            # cc stores on the idle Sync queue
                            nc.sync.dma_start(
                                wo_t[:, qb0 * 2:(qb0 + 1) * 2, :],
                                wo_d[:, qb0 * 2:(qb0 + 1) * 2, :])

                    for qb in range(S // 512):
                        nkc = 4 * (qb + 1)  # causal: only k <= q blocks
                        npair = nkc // 2
                        q_sl = slice(qb * 512, (qb + 1) * 512)
                        at_ps = ps_at.tile([P, 512], F32, name="at_ps")
                        # two half-accumulators side by side: one [128,1024]
                        # vector op per pair instead of two [128,512] ops
                        den_bf = att_sb.tile([P, 2, 512], BF16, name="den_bf")
                        pts = [None] * npair

                        def issue_pair(pr):
                            s_ps = ps_s.tile([P, 2, 512], F32, name="s_ps")
                            for u in range(2):
                                kc = 2 * pr + u
                                nc.tensor.matmul(
                                    s_ps[:, u, :],
                                    KT[h][:, kc * P:(kc + 1) * P],
                                    QT[h][:, q_sl], start=True, stop=True)
                            pt = pt_pool.tile([P, 2, 512], BF16, name="pt")
                            nc.scalar.activation(pt[:], s_ps[:], AF.Exp)
                            if pr >= 2 * qb:
                                # diagonal pair: zero the strictly-upper parts
                                i0 = 2 * (pr - 2 * qb)
                                nc.vector.tensor_mul(
                                    pt[:], pt[:], masks[:, i0:i0 + 2, :])
                            # denominator chunk partial sums (vector, bf16)
                            if pr == 0:
                                nc.vector.tensor_scalar_add(den_bf[:], pt[:],
                                                            0.0)
                            else:
                                nc.vector.tensor_add(den_bf[:], den_bf[:],
                                                     pt[:])
                            pts[pr] = pt

                        for pr in range(min(2, npair)):
                            issue_pair(pr)
                        # previous qb's normalize runs here, off the PE's
                        # critical path (its den chain finished long ago)
                        flush_norm()
                        for pr in range(npair):
                            if pr + 2 < npair:
                                issue_pair(pr + 2)
                            for u in range(2):
                                kc = 2 * pr + u
                                st, sp = kc == 0, kc == nkc - 1
                                nc.tensor.matmul(
                                    at_ps[:], V_t[:, kc, h * DH:(h + 1) * DH],
                                    pts[pr][:, u, :], start=st, stop=sp)
                        pending[0] = (qb, at_ps, den_bf)
                    flush_norm()
                # redistribute this head's outputs; the next head's QKV
                # pass + attention run under this collective
                nc.gpsimd.collective_compute(
                    "AllToAll", mybir.AluOpType.bypass,
                    replica_groups=[CORE_IDS],
                    ins=[cc_in[h][:]], outs=[cc_out[h][:]])

            for h in range(HPC):
                qkv_pass(h)
                if h == 0:
                    # prefetch pass-1's first x tiles now so they land on the
                    # Sync queue ahead of attention-0's cc stores
                    for sbi, dc4 in [(0, 0), (0, 1), (0, 2), (0, 3)]:
                        prefetched[(sbi, dc4)] = prefetch_xt(sbi, dc4)
                attention(h)

            # ---------------- Phase 3: output projection ----------------
            with (
                tc.tile_pool(name="osb", bufs=3) as osb,
                tc.tile_pool(name="ps_o", bufs=1, space="PSUM") as ps_o,
            ):
                # cc_out[h][j, p, q] = attn^T for global head (2j+h), own q
                # slice. Two tiles per head (j halves) so the j<4 matmuls can
                # start while the second half is still loading.
                # aT loads stay on Sync only: putting them on the scalar
                # queue head-of-line blocks attention work behind the
                # collective wait, and any gpsimd (software-DGE) DMA in
                # flight during an AllToAll roughly doubles its duration
                # (descgen contention with NRT's collective processing).
                NH = NCORES // 2
                aT = [[p3.tile([P, NH, QROWS], BF16, name=f"aT{h}_{half}")
                       for half in range(2)] for h in range(HPC)]
                for h in range(HPC):
                    for half in range(2):
                        if h == 1:
                            # h1 is on the critical path after AllToAll#1:
                            # quarter-granularity loads let the first
                            # output-projection matmuls start ~1.5us sooner
                            for q4 in range(2):
                                j0 = half * NH + q4 * (NH // 2)
                                nc.sync.dma_start(
                                    aT[h][half][:, q4 * (NH // 2):
                                                 (q4 + 1) * (NH // 2), :],
                                    cc_out[h][j0:j0 + NH // 2]
                                    .rearrange("j p q -> p j q"))
                        else:
                            nc.sync.dma_start(
                                aT[h][half][:],
                                cc_out[h][half * NH:(half + 1) * NH]
                                .rearrange("j p q -> p j q"))

                # all 8 (qc, fb) groups live in 8 PSUM banks at once; all
                # head-0 contributions (available after the first AllToAll)
                # run first, overlapping the second AllToAll
                blocks = [(qc, fb) for qc in range(QROWS // P)
                          for fb in range(D // 512)]
                o_ps = {b: ps_o.tile([P, 512], F32, name=f"o_ps_{b[0]}_{b[1]}")
                        for b in blocks}
                for h in range(HPC):
                    for half in range(2):
                        for qc, fb in blocks:
                            for jj in range(NH):
                                j = half * NH + jj
                                nc.tensor.matmul(
                                    o_ps[(qc, fb)][:],
                                    aT[h][half][:, jj, qc * P:(qc + 1) * P],
                                    wo_t[:, 2 * j + h,
                                         fb * 512:(fb + 1) * 512],
                                    start=(h == 0 and j == 0),
                                    stop=(h == HPC - 1 and j == NCORES - 1))
                for gi, (qc, fb) in enumerate(blocks):
                    o_sb = osb.tile([P, 512], F32, name="o_sb")
                    nc.vector.tensor_add(o_sb[:], o_ps[(qc, fb)][:],
                                         bo_t[:, fb * 512:(fb + 1) * 512])
                    # alternate store queues so the 2MB of output doesn't
                    # serialize on one DMA ring at the very end
                    eng = nc.sync if gi % 2 == 0 else nc.scalar
                    eng.dma_start(
                        out_d[qc * P:(qc + 1) * P, fb * 512:(fb + 1) * 512],
                        o_sb[:])
            xt_pool.release()
            wpool.release()
            p3.release()

    nc.finalize()
    return nc


def kernel(x, mask, Wq, bq, Wk, bk, Wv, bv, Wo, bo):
    """Full-input MHA forward. Returns the full (2048, 2048) fp32 output.

    The mask input is assumed to be the strictly-upper-triangular causal mask
    the reference generates; causality is applied structurally on-device.
    """
    global LAST_RESULTS
    if "nc" not in _CACHE:
        _CACHE["nc"] = _build_module()
    nc = _CACHE["nc"]

    x = np.asarray(x, dtype=np.float32)
    Wq = np.asarray(Wq, dtype=np.float32)
    Wk = np.asarray(Wk, dtype=np.float32)
    Wv = np.asarray(Wv, dtype=np.float32)
    Wo = np.asarray(Wo, dtype=np.float32)
    bq = np.asarray(bq, dtype=np.float32)
    bk = np.asarray(bk, dtype=np.float32)
    bv = np.asarray(bv, dtype=np.float32)
    bo = np.asarray(bo, dtype=np.float32)

    bf = ml_dtypes.bfloat16
    xT = np.ascontiguousarray(x.T).astype(bf)
    # woT[p, ec, f] = Wo^T[ec*128 + p, f]
    woT_bf = np.ascontiguousarray(
        Wo.T.astype(bf).reshape(H, P, D).transpose(1, 0, 2))
    bo_row_h = np.ascontiguousarray(bo.reshape(1, D))

    def tile_w(W):  # [EL, D] slice -> [P, D//P, EL] with w[p, dc, e]
        return np.ascontiguousarray(
            W.T.astype(bf).reshape(D // P, P, EL).transpose(1, 0, 2))

    in_maps = []
    for c in range(NCORES):
        e_sl = slice(c * EL, (c + 1) * EL)
        in_maps.append({
            "xT": xT,
            "wq": tile_w(Wq[e_sl, :]),
            "wk": tile_w(Wk[e_sl, :]),
            "wv": tile_w(Wv[e_sl, :]),
            # bias layout [dh, head]; Q bias pre-scaled by 1/sqrt(dh)
            "bq": np.ascontiguousarray((bq[e_sl] * INV_SQRT_DH).reshape(HPC, P).T),
            "bk": np.ascontiguousarray(bk[e_sl].reshape(HPC, P).T),
            "bv": np.ascontiguousarray(np.broadcast_to(bv[e_sl], (P, EL))),
            "wo": woT_bf,
            "bo": bo_row_h,
        })

    res = run_bass_kernel_spmd(nc, in_maps, CORE_IDS)
    LAST_RESULTS = res
    return np.concatenate([res.results[c]["out"] for c in range(NCORES)], axis=0)
